# revision 9
# baseline (speedup 1.0000x reference)
"""Two-layer LSTM (H=51) over [B=4096, T=256] on 8 NeuronCores.

Strategy: data-parallel over batch (512 per core). Per core, a skewed
software pipeline over T+2 phases: phase q computes layer-1 of step q,
layer-2 of step q-1, and the linear head of step q-2.

All sigmoids are rewritten as tanh (sigma(z) = (tanh(z/2)+1)/2) with the
1/2 folded into host-precomputed weights, and states stored doubled
(ht = 2h, ct = 2c).

Both layers' gate matmuls are fused into ONE matmul per gate bank: they
share the same rhs (stk) and their lhsT column blocks target disjoint
output rows (l1 -> 0..50, l2 -> 64..114). Phase 0's spurious l2 output
is cancelled by re-zeroing h2/c2 right after phase 0.

Per phase each group needs one big tanh (ACT), one tanh(c/2) (ACT), and
four fused scalar_tensor_tensor combines split across DVE (u, ht) and
the otherwise-idle Pool engine (v, ct).
"""

import numpy as np

H = 51
T_FULL = 256
B_FULL = 4096
N_CORES = 8

# Stk partition layout (stacked matmul rhs):
#   rows 0..50   : ht1 (= 2*h1)
#   rows 51..63  : junk (zero, weighted by zero)
#   rows 64..114 : ht2 (= 2*h2)
#   row 115      : ones (bias row, DMA-initialized)
#   row 116      : x_t (DMA per step)
ROW_H1 = 0
ROW_JUNK = 51
ROW_H2 = 64
ROW_ONES = 115
ROW_X = 116
K_STK = 117
# gate-row space of the elementwise ops: rows 0..50 layer1, 51..63 junk,
# 64..114 layer2
GP = 115

MW = GP  # matmul output width (zero-padded gate lhsT columns)


def _build_weights(W_ih1, W_hh1, b_ih1, b_hh1, W_ih2, W_hh2, b_ih2, b_hh2,
                   W_lin, b_lin):
    """Host-side packing of lhsT weight tiles.

    Returns WG [K_STK, 4*MW + 1] float32. Four fused gate lhsTs of width
    MW=115 (banks i, g, f, o), each combining layer-1 (output rows 0..50:
    W_hh1 at h1 rows, W_ih1 at the x row, b1 at the ones row) and layer-2
    (output rows 64..114: W_ih2 at h1 rows, W_hh2 at h2 rows, b2 at ones).
    Column 4*MW rows 64..115: [0.5*W_lin; b_lin] for the out head
    (lhsT partitions must match its rhs Stk[64:116] = [ht2; ones]).
    Gate scaling: sigma-gates (i,f,o) rows scaled by 0.5 (tanh(z/2) trick);
    h inputs scaled by 0.5 (states stored doubled).
    """
    b1 = (b_ih1 + b_hh1).astype(np.float64)
    b2 = (b_ih2 + b_hh2).astype(np.float64)
    # reference gate order in the stacked 4H rows: i, f, g, o
    idx = {"i": np.arange(0, H), "f": np.arange(H, 2 * H),
           "g": np.arange(2 * H, 3 * H), "o": np.arange(3 * H, 4 * H)}
    # bank order: f, i, g, o — the f bank is matmul'd and tanh'd FIRST so
    # u=(tf+1)*ct can start while tanh over (i,g,o) still runs; i,g,o stay
    # contiguous for one fused tanh op.
    order = ["f", "i", "g", "o"]
    WG = np.zeros((K_STK, 4 * MW + 1), dtype=np.float64)
    for xi, gate in enumerate(order):
        r = idx[gate]
        s = 0.5 if gate in ("i", "f", "o") else 1.0
        c0 = xi * MW
        # layer 1 (output rows 0..50): z1 = W_ih1 @ x + b1 + W_hh1 @ h1
        col1 = slice(c0, c0 + H)
        WG[ROW_ONES, col1] = s * b1[r]
        WG[ROW_H1:ROW_H1 + H, col1] = s * 0.5 * W_hh1[r, :].T
        WG[ROW_X, col1] = s * W_ih1[r, 0]
        # layer 2 (output rows 64..114): z2 = W_ih2 @ h1 + b2 + W_hh2 @ h2
        col2 = slice(c0 + ROW_H2, c0 + ROW_H2 + H)
        WG[ROW_ONES, col2] = s * b2[r]
        WG[ROW_H1:ROW_H1 + H, col2] = s * 0.5 * W_ih2[r, :].T
        WG[ROW_H2:ROW_H2 + H, col2] = s * 0.5 * W_hh2[r, :].T
    # out head: lhsT must sit at the same partitions as its rhs Stk[64:116]
    # (= [ht2 (51); ones]), so W_lin goes at rows 64..114 and b_lin at 115.
    WG[ROW_H2:ROW_H2 + H, 4 * MW] = 0.5 * W_lin[0, :]
    WG[ROW_ONES, 4 * MW] = float(np.asarray(b_lin).reshape(-1)[0])
    return np.ascontiguousarray(WG).astype(np.float32)


def build_core_kernel(T, B, groups=2, use_f32r=True):
    """Build the per-core Bass kernel. Inputs: xT [T+1, B], WG [K_STK, 461].
    Output: out_bt [B, T] (full linear head incl. b_lin)."""
    import concourse.bacc as bacc
    import concourse.mybir as mybir
    from concourse.tile import TileContext

    fp = mybir.dt.float32
    fpr = mybir.dt.float32r if use_f32r else fp
    Bg = B // groups

    nc = bacc.Bacc("TRN2", target_bir_lowering=False, debug=False)
    # xT row 0 is a host-prepended row of ones (feeds the bias row of Stk);
    # rows 1..T are input.T
    xT = nc.dram_tensor("xT", [T + 1, B], fpr, kind="ExternalInput")
    WG = nc.dram_tensor("WG", [K_STK, 4 * MW + 1], fpr, kind="ExternalInput")
    out_bt = nc.dram_tensor("out_bt", [B, T], fp, kind="ExternalOutput")

    C = min(128, T)  # output columns buffered in PSUM between flushes
    assert T % C == 0
    assert (B // groups) % 128 == 0, "batch per group must be a multiple of 128"

    with TileContext(nc) as tc:
        with (
            tc.tile_pool(name="persist", bufs=1) as persist,
            tc.tile_pool(name="gpsum", bufs=1, space="PSUM") as gpsum,
            tc.tile_pool(name="opsum", bufs=1, space="PSUM") as opsum,
            tc.tile_pool(name="temps", bufs=3) as temps,
            tc.tile_pool(name="ostage", bufs=2) as ostage,
        ):
            wg = persist.tile([K_STK, 4 * MW + 1], fpr)
            nc.sync.dma_start(out=wg, in_=WG[:, :])

            nchunk = Bg // 128
            stks, cts, gps, pos = [], [], [], []
            for g in range(groups):
                stk = persist.tile([K_STK, Bg], fpr, tag=f"stk{g}")
                ct = persist.tile([GP, Bg], fp, tag=f"ct{g}")
                gp = gpsum.tile([GP, 4 * Bg], fp, tag=f"gp{g}")
                # DVE memset can't target f32r directly; write zero bits
                # through an f32 view (0.0 is exact in f32r).
                nc.vector.memset(stk[:, :].bitcast(fp), 0.0)
                nc.sync.dma_start(out=stk[ROW_ONES:ROW_ONES + 1, :],
                                  in_=xT[0:1, g * Bg:(g + 1) * Bg])
                nc.vector.memset(ct[:, :], 0.0)
                stks.append(stk)
                cts.append(ct)
                gps.append(gp)
                pos.append(opsum.tile([128, nchunk * C], fp, tag=f"po{g}",
                                      name=f"po{g}"))

            add = mybir.AluOpType.add
            mult = mybir.AluOpType.mult
            tanh = mybir.ActivationFunctionType.Tanh

            for q in range(T + 2):
                mm = q <= T
                # ---- x load for step q + fused gate matmuls (all 4 banks,
                # both layers in one matmul each; at q==T the l1 half reads
                # stale x and produces garbage h1_T/c1_T, which nothing
                # consumes).
                for g in range(groups):
                    stk, gp = stks[g], gps[g]
                    cols = slice(g * Bg, (g + 1) * Bg)
                    if q < T:
                        nc.sync.dma_start(out=stk[ROW_X:ROW_X + 1, :],
                                          in_=xT[q + 1:q + 2, cols])
                    if mm:
                        rhs = stk[0:K_STK, :]
                        for xi in range(4):
                            nc.tensor.matmul(
                                gp[0:GP, xi * Bg:(xi + 1) * Bg],
                                wg[0:K_STK, xi * MW:(xi + 1) * MW],
                                rhs, start=True, stop=True)
                # ---- out head for step t = q-2: out[:, t] column
                if q >= 2:
                    t = q - 2
                    tc_col = t % C
                    for g in range(groups):
                        stk = stks[g]
                        for k in range(nchunk):
                            # f32r rejects N=1 matmuls; run the tiny out
                            # head in plain f32 via bitcast views.
                            nc.tensor.matmul(
                                pos[g][:, k * C + tc_col:k * C + tc_col + 1],
                                stk[64:116, k * 128:(k + 1) * 128].bitcast(fp),
                                wg[64:116, 4 * MW:4 * MW + 1].bitcast(fp),
                                start=True, stop=True)
                    if tc_col == C - 1:  # flush epoch
                        t0 = t - (C - 1)
                        for g in range(groups):
                            for k in range(nchunk):
                                st = ostage.tile([128, C], fp, tag=f"os{g}_{k}")
                                nc.vector.tensor_copy(st, pos[g][:, k * C:(k + 1) * C])
                                row0 = g * Bg + k * 128
                                nc.sync.dma_start(
                                    out=out_bt[row0:row0 + 128, t0:t0 + C],
                                    in_=st)
                # ---- elementwise chain, emitted per group as a full chain:
                # in steady state the two groups run half a period out of
                # phase, so each engine's in-order queue sees its ops in the
                # order they become ready. Banks: 0=f, 1=i, 2=g, 3=o.
                # tanh is split: the f bank first (small op), then i,g,o in
                # one op — u=(tf+1)*ct runs on DVE under the big tanh.
                if mm:
                    for g in range(groups):
                        tf_t = temps.tile([GP, Bg], fp, tag=f"tf{g}")
                        tio_t = temps.tile([GP, 3 * Bg], fp, tag=f"tio{g}")
                        nc.scalar.activation(tf_t, gps[g][0:GP, 0:Bg], tanh)
                        nc.scalar.activation(tio_t, gps[g][0:GP, Bg:4 * Bg],
                                             tanh)
                        tf = tf_t[:, :]
                        ti = tio_t[:, 0 * Bg:1 * Bg]
                        tg = tio_t[:, 1 * Bg:2 * Bg]
                        to = tio_t[:, 2 * Bg:3 * Bg]
                        v = temps.tile([GP, Bg], fp, tag=f"v{g}")
                        u = temps.tile([GP, Bg], fp, tag=f"u{g}")
                        tcl = temps.tile([GP, Bg], fp, tag=f"tc{g}")
                        # u = (tf+1)*ct ; v = (ti+1)*tg ; ct = 0.5*u + v
                        nc.vector.scalar_tensor_tensor(u, tf, 1.0, cts[g][:, :],
                                                       add, mult)
                        nc.vector.scalar_tensor_tensor(v, ti, 1.0, tg, add, mult)
                        nc.vector.scalar_tensor_tensor(cts[g][:, :], u, 0.5,
                                                       v, mult, add)
                        # tanh(c) = tanh(0.5*ct); ht = (to+1)*tanh(c)
                        nc.scalar.activation(tcl, cts[g][:, :], tanh, scale=0.5)
                        nc.vector.scalar_tensor_tensor(
                            stks[g][ROW_H1:ROW_H1 + GP, :], to, 1.0, tcl,
                            add, mult)
                if q == 0:
                    # cancel phase 0's spurious l2 output: h2/c2 must enter
                    # phase 1 as zero.
                    for g in range(groups):
                        nc.vector.memset(
                            stks[g][ROW_H2:ROW_H2 + H, :].bitcast(fp), 0.0)
                        nc.vector.memset(cts[g][ROW_H2:ROW_H2 + H, :], 0.0)
    nc.compile()
    return nc


_NC_CACHE = {}


def _get_nc(T, B, groups, use_f32r):
    key = (T, B, groups, use_f32r)
    if key not in _NC_CACHE:
        _NC_CACHE[key] = build_core_kernel(T, B, groups, use_f32r)
    return _NC_CACHE[key]


def kernel(input, W_ih1, W_hh1, b_ih1, b_hh1, W_ih2, W_hh2, b_ih2, b_hh2,
           W_lin, b_lin, _groups=2, _use_f32r=True):
    from concourse import bass_utils

    input = np.asarray(input, dtype=np.float32)
    B, T = input.shape
    Bc = B // N_CORES
    WG = _build_weights(np.asarray(W_ih1, np.float64), np.asarray(W_hh1, np.float64),
                        np.asarray(b_ih1, np.float64), np.asarray(b_hh1, np.float64),
                        np.asarray(W_ih2, np.float64), np.asarray(W_hh2, np.float64),
                        np.asarray(b_ih2, np.float64), np.asarray(b_hh2, np.float64),
                        np.asarray(W_lin, np.float64), np.asarray(b_lin, np.float64))
    # row 0 = ones (bias row), rows 1..T = input.T
    xT = np.concatenate([np.ones((1, B), np.float32), input.T.astype(np.float32)])
    nc = _get_nc(T, Bc, _groups, _use_f32r)
    in_maps = [
        {"xT": np.ascontiguousarray(xT[:, c * Bc:(c + 1) * Bc]), "WG": WG}
        for c in range(N_CORES)
    ]
    res = bass_utils.run_bass_kernel_spmd(
        nc, in_maps, core_ids=list(range(N_CORES)), trace=False)
    outs = [res.results[c]["out_bt"] for c in range(N_CORES)]  # [Bc, T] each
    out = np.concatenate(outs, axis=0)  # [B, T]
    return out.astype(np.float32)


# revision 10
# speedup vs baseline: 1.1182x; 1.1182x over previous
"""Two-layer LSTM (H=51) over [B=4096, T=256] on 8 NeuronCores.

Strategy: data-parallel over batch (512 per core). Per core, a skewed
software pipeline over T+2 phases: phase q computes layer-1 of step q,
layer-2 of step q-1, and the linear head of step q-2.

All sigmoids are rewritten as tanh (sigma(z) = (tanh(z/2)+1)/2) with the
1/2 folded into host-precomputed weights, and states stored doubled
(ht = 2h, ct = 2c).

Both layers' gate matmuls are fused into ONE matmul per gate bank: they
share the same rhs (stk) and their lhsT column blocks target disjoint
output rows (l1 -> 0..50, l2 -> 64..114). Phase 0's spurious l2 output
is cancelled by re-zeroing h2/c2 right after phase 0.

Per phase each group needs one big tanh (ACT), one tanh(c/2) (ACT), and
four fused scalar_tensor_tensor combines split across DVE (u, ht) and
the otherwise-idle Pool engine (v, ct).
"""

import numpy as np

H = 51
T_FULL = 256
B_FULL = 4096
N_CORES = 8

# Stk partition layout (stacked matmul rhs):
#   rows 0..50   : ht1 (= 2*h1)
#   rows 51..63  : junk (zero, weighted by zero)
#   rows 64..114 : ht2 (= 2*h2)
#   row 115      : ones (bias row, DMA-initialized)
#   row 116      : x_t (DMA per step)
ROW_H1 = 0
ROW_JUNK = 51
ROW_H2 = 64
ROW_ONES = 115
ROW_X = 116
K_STK = 117
# gate-row space of the elementwise ops: rows 0..50 layer1, 51..63 junk,
# 64..114 layer2
GP = 115

MW = GP  # matmul output width (zero-padded gate lhsT columns)


def _build_weights(W_ih1, W_hh1, b_ih1, b_hh1, W_ih2, W_hh2, b_ih2, b_hh2,
                   W_lin, b_lin):
    """Host-side packing of lhsT weight tiles.

    Returns WG [K_STK, 4*MW + 1] float32. Four fused gate lhsTs of width
    MW=115 (banks i, g, f, o), each combining layer-1 (output rows 0..50:
    W_hh1 at h1 rows, W_ih1 at the x row, b1 at the ones row) and layer-2
    (output rows 64..114: W_ih2 at h1 rows, W_hh2 at h2 rows, b2 at ones).
    Column 4*MW rows 64..115: [0.5*W_lin; b_lin] for the out head
    (lhsT partitions must match its rhs Stk[64:116] = [ht2; ones]).
    Gate scaling: sigma-gates (i,f,o) rows scaled by 0.5 (tanh(z/2) trick);
    h inputs scaled by 0.5 (states stored doubled).
    """
    b1 = (b_ih1 + b_hh1).astype(np.float64)
    b2 = (b_ih2 + b_hh2).astype(np.float64)
    # reference gate order in the stacked 4H rows: i, f, g, o
    idx = {"i": np.arange(0, H), "f": np.arange(H, 2 * H),
           "g": np.arange(2 * H, 3 * H), "o": np.arange(3 * H, 4 * H)}
    # bank order: f, i, g, o — the f bank is matmul'd and tanh'd FIRST so
    # u=(tf+1)*ct can start while tanh over (i,g,o) still runs; i,g,o stay
    # contiguous for one fused tanh op.
    order = ["f", "i", "g", "o"]
    WG = np.zeros((K_STK, 4 * MW + 1), dtype=np.float64)
    for xi, gate in enumerate(order):
        r = idx[gate]
        s = 0.5 if gate in ("i", "f", "o") else 1.0
        c0 = xi * MW
        # layer 1 (output rows 0..50): z1 = W_ih1 @ x + b1 + W_hh1 @ h1
        col1 = slice(c0, c0 + H)
        WG[ROW_ONES, col1] = s * b1[r]
        WG[ROW_H1:ROW_H1 + H, col1] = s * 0.5 * W_hh1[r, :].T
        WG[ROW_X, col1] = s * W_ih1[r, 0]
        # layer 2 (output rows 64..114): z2 = W_ih2 @ h1 + b2 + W_hh2 @ h2
        col2 = slice(c0 + ROW_H2, c0 + ROW_H2 + H)
        WG[ROW_ONES, col2] = s * b2[r]
        WG[ROW_H1:ROW_H1 + H, col2] = s * 0.5 * W_ih2[r, :].T
        WG[ROW_H2:ROW_H2 + H, col2] = s * 0.5 * W_hh2[r, :].T
    # out head: lhsT must sit at the same partitions as its rhs Stk[64:116]
    # (= [ht2 (51); ones]), so W_lin goes at rows 64..114 and b_lin at 115.
    WG[ROW_H2:ROW_H2 + H, 4 * MW] = 0.5 * W_lin[0, :]
    WG[ROW_ONES, 4 * MW] = float(np.asarray(b_lin).reshape(-1)[0])
    return np.ascontiguousarray(WG).astype(np.float32)


def build_core_kernel(T, B, groups=2, use_f32r=True):
    """Build the per-core Bass kernel. Inputs: xT [T+1, B], WG [K_STK, 461].
    Output: out_bt [B, T] (full linear head incl. b_lin)."""
    import concourse.bacc as bacc
    import concourse.mybir as mybir
    from concourse.tile import TileContext

    fp = mybir.dt.float32
    fpr = mybir.dt.float32r if use_f32r else fp
    Bg = B // groups

    nc = bacc.Bacc("TRN2", target_bir_lowering=False, debug=False)
    # xT row 0 is a host-prepended row of ones (feeds the bias row of Stk);
    # rows 1..T are input.T
    xT = nc.dram_tensor("xT", [T + 1, B], fpr, kind="ExternalInput")
    WG = nc.dram_tensor("WG", [K_STK, 4 * MW + 1], fpr, kind="ExternalInput")
    out_bt = nc.dram_tensor("out_bt", [B, T], fp, kind="ExternalOutput")

    C = min(128, T)  # output columns buffered in PSUM between flushes
    assert T % C == 0
    assert (B // groups) % 128 == 0, "batch per group must be a multiple of 128"

    with TileContext(nc) as tc:
        with (
            tc.tile_pool(name="persist", bufs=1) as persist,
            tc.tile_pool(name="gpsum", bufs=1, space="PSUM") as gpsum,
            tc.tile_pool(name="opsum", bufs=1, space="PSUM") as opsum,
            tc.tile_pool(name="temps", bufs=3) as temps,
            tc.tile_pool(name="ostage", bufs=2) as ostage,
        ):
            wg = persist.tile([K_STK, 4 * MW + 1], fpr)
            nc.sync.dma_start(out=wg, in_=WG[:, :])

            nchunk = Bg // 128
            stks, cts, gps, pos = [], [], [], []
            for g in range(groups):
                stk = persist.tile([K_STK, Bg], fpr, tag=f"stk{g}")
                ct = persist.tile([GP, Bg], fp, tag=f"ct{g}")
                gp = gpsum.tile([GP, 4 * Bg], fp, tag=f"gp{g}")
                # DVE memset can't target f32r directly; write zero bits
                # through an f32 view (0.0 is exact in f32r).
                nc.vector.memset(stk[:, :].bitcast(fp), 0.0)
                nc.sync.dma_start(out=stk[ROW_ONES:ROW_ONES + 1, :],
                                  in_=xT[0:1, g * Bg:(g + 1) * Bg])
                nc.vector.memset(ct[:, :], 0.0)
                stks.append(stk)
                cts.append(ct)
                gps.append(gp)
                pos.append(opsum.tile([128, nchunk * C], fp, tag=f"po{g}",
                                      name=f"po{g}"))

            add = mybir.AluOpType.add
            mult = mybir.AluOpType.mult
            tanh = mybir.ActivationFunctionType.Tanh

            for q in range(T + 2):
                mm = q <= T
                # ---- x load for step q + fused gate matmuls (all 4 banks,
                # both layers in one matmul each; at q==T the l1 half reads
                # stale x and produces garbage h1_T/c1_T, which nothing
                # consumes).
                for g in range(groups):
                    stk, gp = stks[g], gps[g]
                    cols = slice(g * Bg, (g + 1) * Bg)
                    if q < T:
                        nc.sync.dma_start(out=stk[ROW_X:ROW_X + 1, :],
                                          in_=xT[q + 1:q + 2, cols])
                    if mm:
                        rhs = stk[0:K_STK, :]
                        for xi in range(4):
                            nc.tensor.matmul(
                                gp[0:GP, xi * Bg:(xi + 1) * Bg],
                                wg[0:K_STK, xi * MW:(xi + 1) * MW],
                                rhs, start=True, stop=True)
                # ---- out head for step t = q-2: out[:, t] column
                if q >= 2:
                    t = q - 2
                    tc_col = t % C
                    for g in range(groups):
                        stk = stks[g]
                        for k in range(nchunk):
                            # f32r rejects N=1 matmuls; run the tiny out
                            # head in plain f32 via bitcast views.
                            nc.tensor.matmul(
                                pos[g][:, k * C + tc_col:k * C + tc_col + 1],
                                stk[64:116, k * 128:(k + 1) * 128].bitcast(fp),
                                wg[64:116, 4 * MW:4 * MW + 1].bitcast(fp),
                                start=True, stop=True)
                    if tc_col == C - 1:  # flush epoch
                        t0 = t - (C - 1)
                        for g in range(groups):
                            for k in range(nchunk):
                                st = ostage.tile([128, C], fp, tag=f"os{g}_{k}")
                                nc.vector.tensor_copy(st, pos[g][:, k * C:(k + 1) * C])
                                row0 = g * Bg + k * 128
                                nc.sync.dma_start(
                                    out=out_bt[row0:row0 + 128, t0:t0 + C],
                                    in_=st)
                # ---- elementwise chain, emitted per group as a full chain:
                # in steady state the two groups run half a period out of
                # phase, so each engine's in-order queue sees its ops in the
                # order they become ready. Banks: 0=f, 1=i, 2=g, 3=o.
                # tanh is split: the f bank first (small op), then i,g,o in
                # one op — u=(tf+1)*ct runs on DVE under the big tanh.
                if mm:
                    for g in range(groups):
                        tg_t = temps.tile([GP, 4 * Bg], fp, tag=f"tg{g}")
                        nc.scalar.activation(tg_t, gps[g][0:GP, :], tanh)
                        tf = tg_t[:, 0 * Bg:1 * Bg]
                        ti = tg_t[:, 1 * Bg:2 * Bg]
                        tg = tg_t[:, 2 * Bg:3 * Bg]
                        to = tg_t[:, 3 * Bg:4 * Bg]
                        v = temps.tile([GP, Bg], fp, tag=f"v{g}")
                        u = temps.tile([GP, Bg], fp, tag=f"u{g}")
                        tcl = temps.tile([GP, Bg], fp, tag=f"tc{g}")
                        # u = (tf+1)*ct ; v = (ti+1)*tg ; ct = 0.5*u + v
                        nc.vector.scalar_tensor_tensor(u, tf, 1.0, cts[g][:, :],
                                                       add, mult)
                        nc.vector.scalar_tensor_tensor(v, ti, 1.0, tg, add, mult)
                        nc.vector.scalar_tensor_tensor(cts[g][:, :], u, 0.5,
                                                       v, mult, add)
                        # tanh(c) = tanh(0.5*ct); ht = (to+1)*tanh(c)
                        nc.scalar.activation(tcl, cts[g][:, :], tanh, scale=0.5)
                        nc.vector.scalar_tensor_tensor(
                            stks[g][ROW_H1:ROW_H1 + GP, :], to, 1.0, tcl,
                            add, mult)
                if q == 0:
                    # cancel phase 0's spurious l2 output: h2/c2 must enter
                    # phase 1 as zero.
                    for g in range(groups):
                        nc.vector.memset(
                            stks[g][ROW_H2:ROW_H2 + H, :].bitcast(fp), 0.0)
                        nc.vector.memset(cts[g][ROW_H2:ROW_H2 + H, :], 0.0)
    nc.compile()
    return nc


_NC_CACHE = {}


def _get_nc(T, B, groups, use_f32r):
    key = (T, B, groups, use_f32r)
    if key not in _NC_CACHE:
        _NC_CACHE[key] = build_core_kernel(T, B, groups, use_f32r)
    return _NC_CACHE[key]


def kernel(input, W_ih1, W_hh1, b_ih1, b_hh1, W_ih2, W_hh2, b_ih2, b_hh2,
           W_lin, b_lin, _groups=2, _use_f32r=True):
    from concourse import bass_utils

    input = np.asarray(input, dtype=np.float32)
    B, T = input.shape
    Bc = B // N_CORES
    WG = _build_weights(np.asarray(W_ih1, np.float64), np.asarray(W_hh1, np.float64),
                        np.asarray(b_ih1, np.float64), np.asarray(b_hh1, np.float64),
                        np.asarray(W_ih2, np.float64), np.asarray(W_hh2, np.float64),
                        np.asarray(b_ih2, np.float64), np.asarray(b_hh2, np.float64),
                        np.asarray(W_lin, np.float64), np.asarray(b_lin, np.float64))
    # row 0 = ones (bias row), rows 1..T = input.T
    xT = np.concatenate([np.ones((1, B), np.float32), input.T.astype(np.float32)])
    nc = _get_nc(T, Bc, _groups, _use_f32r)
    in_maps = [
        {"xT": np.ascontiguousarray(xT[:, c * Bc:(c + 1) * Bc]), "WG": WG}
        for c in range(N_CORES)
    ]
    res = bass_utils.run_bass_kernel_spmd(
        nc, in_maps, core_ids=list(range(N_CORES)), trace=False)
    outs = [res.results[c]["out_bt"] for c in range(N_CORES)]  # [Bc, T] each
    out = np.concatenate(outs, axis=0)  # [B, T]
    return out.astype(np.float32)
